# revision 1
# baseline (speedup 1.0000x reference)
"""Trainium2 Bass kernel for DVectorNet (LSTM -> gather last -> MLP head -> softmax).

Self-contained: takes full unsharded inputs, shards batch across 8 NeuronCores,
compiles+runs a Bass/Tile kernel via concourse.bass_utils.run_bass_kernel_spmd,
gathers the full output.

Per-core layout (state-transposed, [feature, batch]):
  - rhs staging tile xs [128, G*BL]: rows 0-40 = x_t + ones feature (DMA,
    G steps per refill), rows 41-63 = zero, rows 64-127 = h_t (written by
    the h op of the previous step). One K=128 matmul per gate block.
  - gate blocks ZA = [i; f], ZB = [j; o] in PSUM; sigmoid(ZA) in place,
    tanh(j)/sigmoid(o) to SBUF; forget bias folded into the bias row.
  - base-partition rule (SB+SB ops must share start partition): i/f/j/c/u/v
    at base 0; o/tanh_c/h/mask/h-final at base 64.

Length-sorted shrinking batch: samples are sorted by seq_length descending
and dealt round-robin to cores, so all cores share one width schedule
w_t = ceil(#{len > t} / 8).  Every per-step instruction is sliced to its
active width, so per-step work decays as sequences finish; the last valid
h per sample is still captured exactly via the per-step predicated copy
(a sample's column stays active through its own last step since the global
schedule is an upper bound on each core's active count). The program is
specialized to the width schedule (cached per seq_length multiset).
"""

import numpy as np

B, T, D, H = 2048, 300, 40, 64
D1, OUT = 128, 512
NCORES = 8
BL = B // NCORES  # 256 samples per core
FORGET_BIAS = 1.0
BN_EPS = 1e-3
G = 10  # steps per x/mask DMA refill group

_cache = {}


def _split_waits(nc, mybir, maxw=1):
    """Walrus in this env rejects instructions with >maxw semaphore waits;
    move excess waits onto NoOp instructions just before the offender."""
    ctr = 0
    for f in nc.m.functions:
        for bb in f.blocks:
            out, changed = [], False
            for inst in bb.instructions:
                si = inst.sync_info
                waits = list(si.on_wait) if si is not None else []
                if len(waits) > maxw:
                    changed = True
                    extra, keep = waits[:-maxw], waits[-maxw:]
                    for i in range(0, len(extra), maxw):
                        nop = mybir.InstNoOp(
                            name=f"waitnop-{ctr}", engine=inst.engine,
                            sync_info=mybir.SyncInfo(
                                on_wait=list(extra[i:i + maxw]), on_update=[]))
                        ctr += 1
                        nc.register_instruction(nop, overwrite=True)
                        out.append(nop)
                    inst.sync_info = mybir.SyncInfo(
                        on_wait=list(keep), on_update=list(si.on_update))
                out.append(inst)
            if changed:
                bb.instructions = out


def _build_nc(t_steps, widths=None, loop_steps=None, reps=1):
    import concourse.bass as bass
    import concourse.mybir as mybir
    from concourse import tile

    f32 = mybir.dt.float32
    f32r = mybir.dt.float32r
    AF = mybir.ActivationFunctionType

    if loop_steps is None:
        loop_steps = t_steps
    if widths is None:
        widths = [BL] * t_steps
    ngrp = (t_steps + G - 1) // G

    nc = bass.Bass()
    xt = nc.dram_tensor("xt", [ngrp, D + 1, G * BL], f32r, kind="ExternalInput")
    mb = nc.dram_tensor("mb", [ngrp, H, G * BL], mybir.dt.uint8, kind="ExternalInput")
    wg = nc.dram_tensor("wg", [2, 128, 128], f32r, kind="ExternalInput")
    w1 = nc.dram_tensor("w1", [128, D1], f32, kind="ExternalInput")
    w2 = nc.dram_tensor("w2", [D1, OUT], f32, kind="ExternalInput")
    b2 = nc.dram_tensor("b2", [1, OUT], f32, kind="ExternalInput")
    out = nc.dram_tensor("out", [BL, OUT], f32, kind="ExternalOutput")

    with tile.TileContext(nc) as tc:
        with (
            tc.tile_pool(name="const", bufs=1) as cpool,
            tc.tile_pool(name="state", bufs=1) as spool,
            tc.tile_pool(name="psum", bufs=1, space="PSUM") as ppool,
        ):
            # --- persistent weights ---
            wg_s = [cpool.tile([128, 128], f32r, name=f"wg{b}") for b in range(2)]
            w1_s = cpool.tile([128, D1], f32, name="w1s")
            w2_s = cpool.tile([D1, OUT], f32, name="w2s")
            b2_s = cpool.tile([1, OUT], f32, name="b2s")
            ones_c = cpool.tile([1, 128], f32, name="onesc")
            for b in range(2):
                nc.sync.dma_start(out=wg_s[b][:], in_=wg[b])
            nc.sync.dma_start(out=w1_s[:], in_=w1[:])
            nc.sync.dma_start(out=w2_s[:], in_=w2[:])
            nc.sync.dma_start(out=b2_s[:], in_=b2[:])
            nc.vector.memset(ones_c[:], 1.0)

            # --- state tiles ---
            xs = [spool.tile([128, G * BL], f32r, name=f"xs{i}") for i in range(2)]
            ms = [spool.tile([128, G * BL], mybir.dt.uint8, name=f"ms{i}") for i in range(2)]
            cst = [spool.tile([H, BL], f32, name=f"c{i}") for i in range(2)]
            S_g = [spool.tile([128, BL], f32, name=f"sg{i}") for i in range(2)]
            T_c = [spool.tile([128, BL], f32, name=f"tc{i}") for i in range(2)]
            u_t = spool.tile([H, BL], f32, name="u")
            v_t = spool.tile([H, BL], f32, name="v")
            hfin = spool.tile([128, BL], f32, name="hfin")

            ZA = [ppool.tile([128, BL], f32, name=f"ZA{i}") for i in range(2)]
            ZB = [ppool.tile([128, BL], f32, name=f"ZB{i}") for i in range(2)]

            # --- init ---
            nc.vector.memset(xs[0][32:64, :].bitcast(f32), 0.0)  # zero rows 41-63
            nc.vector.memset(xs[1][32:64, :].bitcast(f32), 0.0)
            nc.vector.memset(xs[0][64:128, 0:BL].bitcast(f32), 0.0)  # h0 = 0
            nc.vector.memset(cst[0][:], 0.0)            # c0 = 0
            nc.vector.memset(hfin[0:64, :], 0.0)
            nc.vector.memset(hfin[0:1, :], 1.0)         # ones row for head bias
            nc.gpsimd.memset(hfin[64:128, :], 0.0)      # h accumulation rows
            nc.sync.dma_start(out=xs[0][0:D + 1, :], in_=xt[0])
            nc.sync.dma_start(out=ms[0][64:128, :], in_=mb[0])

            # --- LSTM loop (optionally repeated for timing runs) ---
            import contextlib
            rep_ctx = tc.For_i(0, reps, 1) if reps > 1 else contextlib.nullcontext()
            with rep_ctx:
              for t in range(loop_steps):
                  g, sl = t // G, t % G
                  cur, nxt = g % 2, (g + 1) % 2
                  t1 = t + 1
                  g1, sl1 = t1 // G, t1 % G
                  cur1 = g1 % 2
                  w = widths[t]
                  w1n = widths[t1] if t1 < t_steps else 0
                  cb = slice(sl * BL, sl * BL + w)
                  cb1 = slice(sl1 * BL, sl1 * BL + w1n)
                  if sl == 0 and g + 1 < ngrp:
                      # prefetch next group's x and mask
                      nc.sync.dma_start(out=xs[nxt][0:D + 1, :], in_=xt[g + 1])
                      nc.sync.dma_start(out=ms[nxt][64:128, :], in_=mb[g + 1])

                  tp = t % 2
                  # gate blocks: ZA = [i; f] (stays in PSUM, sigmoid in place),
                  # ZB = [j; o] (tanh/sigmoid to SBUF). fp32r matmuls: 1 cyc/row.
                  nc.tensor.matmul(ZA[tp][:, 0:w], wg_s[0][:], xs[cur][:, cb],
                                   start=True, stop=True)
                  nc.tensor.matmul(ZB[tp][:, 0:w], wg_s[1][:], xs[cur][:, cb],
                                   start=True, stop=True)
                  nc.scalar.activation(ZA[tp][:, 0:w], ZA[tp][:, 0:w], AF.Sigmoid)
                  nc.scalar.activation(S_g[tp][0:64, 0:w], ZB[tp][0:64, 0:w],
                                       AF.Tanh)
                  nc.scalar.activation(S_g[tp][64:128, 0:w],
                                       ZB[tp][64:128, 0:w], AF.Sigmoid)
                  psi = ZA[tp][0:64, 0:w]    # sigmoid(i) in PSUM
                  psf = ZA[tp][64:128, 0:w]  # sigmoid(f) in PSUM
                  tj = S_g[tp][0:64, 0:w]      # tanh(j) in SBUF @0
                  so = S_g[tp][64:128, 0:w]    # sigmoid(o) in SBUF @64
                  # v = sigmoid(f) * c   (PSUM operand exempt from base rule)
                  nc.vector.tensor_mul(v_t[:, 0:w], psf, cst[tp][:, 0:w])
                  # u = tanh(j) * sigmoid(i)
                  nc.vector.tensor_mul(u_t[:, 0:w], tj, psi)
                  nc.vector.tensor_add(cst[(t + 1) % 2][:, 0:w], u_t[:, 0:w],
                                       v_t[:, 0:w])
                  nc.scalar.activation(T_c[tp][64:128, 0:w],
                                       cst[(t + 1) % 2][:, 0:w], AF.Tanh)
                  # h(t) -> rhs rows of next step's matmul input (base 64).
                  # Full active width w so finishers' h exists for capture;
                  # next step only reads its own w1n columns.
                  hcb = slice(sl1 * BL, sl1 * BL + w)
                  nc.vector.tensor_mul(xs[cur1][64:128, hcb],
                                       T_c[tp][64:128, 0:w], so)
                  # capture h(t) where t == seq_length-1
                  nc.vector.copy_predicated(hfin[64:128, 0:w],
                                            ms[cur][64:128, cb],
                                            xs[cur1][64:128, hcb].bitcast(f32))

            # --- head: dense1 + relu (BN folded into w2 on host) ---
            z1 = ppool.tile([D1, BL], f32, name="z1")
            relu1 = spool.tile([D1, BL], f32, name="relu1")
            nc.tensor.matmul(z1[:], w1_s[:], hfin[:], start=True, stop=True)
            nc.scalar.activation(relu1[:], z1[:], AF.Relu)

            # --- head: dense2 + softmax, 2 chunks of 128 samples ---
            for ch in range(2):
                sl2 = slice(ch * 128, (ch + 1) * 128)
                L = ppool.tile([128, OUT], f32, name=f"L{ch}")
                nc.tensor.matmul(L[:], relu1[:, sl2], w2_s[:],
                                 start=True, stop=False)
                nc.tensor.matmul(L[:], ones_c[:], b2_s[:],
                                 start=False, stop=True)
                negmx = spool.tile([128, 1], f32, name=f"negmx{ch}")
                se = spool.tile([128, 1], f32, name=f"se{ch}")
                rse = spool.tile([128, 1], f32, name=f"rse{ch}")
                e_s = spool.tile([128, OUT], f32, name=f"es{ch}")
                o_s = spool.tile([128, OUT], f32, name=f"os{ch}")
                nc.vector.reduce_max(negmx[:], L[:],
                                     axis=mybir.AxisListType.X, negate=True)
                nc.scalar.activation(e_s[:], L[:], AF.Exp,
                                     bias=negmx[:], accum_out=se[:])
                nc.vector.reciprocal(rse[:], se[:])
                nc.vector.tensor_scalar_mul(o_s[:], e_s[:], rse[:])
                nc.sync.dma_start(out=out[sl2, :], in_=o_s[:])

    _split_waits(nc, mybir)
    return nc


def _plan(seq_length, t_steps):
    """Sort samples by length desc, deal round-robin to cores; shared width
    schedule w_t = ceil(#{len > t} / NCORES)."""
    L = np.asarray(seq_length).astype(np.int64)
    order = np.argsort(-L, kind="stable")
    core_samples = [order[k::NCORES] for k in range(NCORES)]
    n_active = (L[None, :] > np.arange(t_steps)[:, None]).sum(axis=1)
    widths = np.minimum(BL, (n_active + NCORES - 1) // NCORES).astype(int)
    widths = np.maximum(widths, 1)
    widths = np.minimum(BL, (widths + 3) // 4 * 4)  # ISA wants aligned matmul cols
    return core_samples, tuple(int(w) for w in widths)


def _prep_inputs(X, seq_length, lstm_kernel, lstm_bias, W1, b1, gamma, beta,
                 moving_mean, moving_var, W2, b2, t_steps, core_samples):
    ngrp = (t_steps + G - 1) // G
    tpad = ngrp * G

    # lstm_kernel [104, 256], gate order i, j, f, o; bias folded via ones
    # feature (row 40 of x). Blocks: ZA = [Wi | Wf], ZB = [Wj | Wo];
    # rows 0-39 Wx, row 40 bias, 41-63 zero, 64-127 Wh.
    Wk = lstm_kernel.astype(np.float64)
    bias_adj = lstm_bias.astype(np.float64).copy()
    bias_adj[2 * H:3 * H] += FORGET_BIAS
    Wi, Wj, Wf, Wo = (Wk[:, 0:H], Wk[:, H:2 * H], Wk[:, 2 * H:3 * H],
                      Wk[:, 3 * H:4 * H])
    bi, bj, bf, bo = (bias_adj[0:H], bias_adj[H:2 * H], bias_adj[2 * H:3 * H],
                      bias_adj[3 * H:4 * H])

    def block(wl, bl_, wr, br):
        blk = np.zeros((128, 128), dtype=np.float64)
        if wl is not None:
            blk[0:D, 0:H] = wl[0:D]
            blk[D, 0:H] = bl_
            blk[64:128, 0:H] = wl[D:D + H]
        if wr is not None:
            blk[0:D, H:128] = wr[0:D]
            blk[D, H:128] = br
            blk[64:128, H:128] = wr[D:D + H]
        return blk

    wg = np.stack([
        block(Wi, bi, Wf, bf),
        block(Wj, bj, Wo, bo),
    ]).astype(np.float32)

    w1 = np.zeros((128, D1), dtype=np.float32)
    w1[0] = b1
    w1[64:128] = W1
    a = gamma.astype(np.float64) / np.sqrt(moving_var.astype(np.float64) + BN_EPS)
    d = beta.astype(np.float64) - a * moving_mean.astype(np.float64)
    w2 = (W2.astype(np.float64) * a[:, None]).astype(np.float32)
    b2f = (b2.astype(np.float64) + d @ W2.astype(np.float64)).astype(np.float32)
    b2f = np.ascontiguousarray(b2f[None, :])

    in_maps = []
    for k in range(NCORES):
        idx = core_samples[k]
        nk = len(idx)
        # x + ones feature, padded to tpad steps, grouped [ngrp, 41, G*BL]
        xk = np.zeros((tpad, D + 1, BL), dtype=np.float32)
        xk[:t_steps, 0:D, 0:nk] = X[idx, :t_steps, :].transpose(1, 2, 0)
        xk[:t_steps, D] = 1.0
        xk = np.ascontiguousarray(
            xk.reshape(ngrp, G, D + 1, BL).transpose(0, 2, 1, 3)
            .reshape(ngrp, D + 1, G * BL))
        sk = seq_length[idx].astype(np.int64)
        m2 = np.zeros((tpad, BL), dtype=bool)
        m2[:, 0:nk] = (np.arange(tpad)[:, None] == (sk[None, :] - 1))
        mbk = np.broadcast_to(m2[:, None, :], (tpad, H, BL))
        mbk = np.ascontiguousarray(
            mbk.reshape(ngrp, G, H, BL).transpose(0, 2, 1, 3)
            .reshape(ngrp, H, G * BL)).astype(np.uint8)
        in_maps.append({
            "xt": xk, "mb": mbk, "wg": wg,
            "w1": w1, "w2": w2, "b2": b2f,
        })
    return in_maps


def kernel(X, seq_length, lstm_kernel, lstm_bias, W1, b1, gamma, beta,
           moving_mean, moving_var, W2, b2, _t_steps=None, _want_results=False,
           _trace=False, _loop_steps=None):
    from concourse.bass_utils import run_bass_kernel_spmd

    t_cap = X.shape[1]
    t_steps = _t_steps if _t_steps is not None else t_cap
    t_steps = min(t_steps, t_cap)
    eff_steps = int(min(t_steps, int(np.max(seq_length))))
    core_samples, widths = _plan(seq_length, eff_steps)

    key = (eff_steps, widths, _loop_steps)
    if key not in _cache:
        _cache[key] = _build_nc(eff_steps, list(widths), _loop_steps)
    nc = _cache[key]

    in_maps = _prep_inputs(X, seq_length, lstm_kernel, lstm_bias, W1, b1,
                           gamma, beta, moving_mean, moving_var, W2, b2,
                           eff_steps, core_samples)
    last_exc = None
    for attempt in range(3):
        try:
            res = run_bass_kernel_spmd(nc, in_maps,
                                       core_ids=list(range(NCORES)),
                                       trace=_trace)
            break
        except Exception as e:  # flaky NRT_EXEC_UNIT_UNRECOVERABLE retries
            last_exc = e
    else:
        raise last_exc
    outs = np.empty((B, OUT), dtype=np.float32)
    for k in range(NCORES):
        idx = core_samples[k]
        outs[idx] = res.results[k]["out"][0:len(idx)]
    if _want_results:
        return outs, res
    return outs

